# revision 23
# baseline (speedup 1.0000x reference)
"""Trainium2 Bass kernel for DMCG GNN message passing (B=32 fully-connected
graphs, N=64 nodes, 12 steps).

Strategy: data-parallel over graphs, 4 graphs per NeuronCore.  All state is
kept feature-major (features on SBUF partitions) so every MLP is a plain
chunked matmul, and the edge set is padded to the full 64x64 grid per graph
so gathers (node[ROWS]/node[COLS]) and scatter-sums (segment_sum over
src/dst/graph) become broadcast adds and axis reductions over a regular
[rows, cols] view.  Diagonal (self-loop) lanes are computed but excluded
exactly: triangular masks for the posen input, zero rows in the padded poseg
weight, and an explicit diagonal zeroing of each e1 tile before the
reductions / residual.

Matmuls run as float32r (full PE rate); the tiny distance-matrix matmuls run
as plain float32 for precision (sqrt near zero amplifies rounding).
"""

import numpy as np

import concourse.bass as bass
import concourse.mybir as mybir
import concourse.tile as tile
from concourse import bacc
from concourse.bass_utils import run_bass_kernel_spmd

# ---- problem sizes (hardcoded; must match the grader's reference) ----
B, N, L, H = 32, 64, 256, 512
STEPS = 12
NCORES = 8
GPC = B // NCORES          # graphs per core
NN = N * N                 # padded edges per graph (incl. diagonal)
NE = GPC * NN              # padded edges per core
ET = 512                   # edges per tile
RT = ET // N               # grid rows per tile
NT = NE // ET              # tiles per core
TPG = NN // ET             # tiles per graph

F32R = mybir.dt.float32r
F32 = mybir.dt.float32
AF = mybir.ActivationFunctionType
OP = mybir.AluOpType

# ---------------------------------------------------------------------------
# bias/scale blob layout (shared between host packing and device slicing)
# ---------------------------------------------------------------------------


def _bias_layout():
    names = [("pe_w1", H), ("pe_b1", H), ("pe_b2", H), ("pe_b3", L),
             ("pn_b1", H), ("pn_b2", H), ("pn_b3", L),
             ("pg_b1", H), ("pg_b2", H), ("pg_b3", L), ("e_b3cum", L)]
    for s in range(STEPS):
        for p, dims in (("e", (H, H, L)), ("n", (H, H, L)), ("g", (H, H, L))):
            for i, d in enumerate(dims):
                names.append((f"s{s}_{p}_b{i + 1}", d))
    off, c = {}, 0
    for nm, d in names:
        off[nm] = (c, d // 128)
        c += d // 128
    return off, c


BIAS_OFF, BIAS_COLS = _bias_layout()

# column layout inside the per-step PREW blob ([128, 3072]):
#   W1r chunks (1024) | W1c chunks (1024) | W1g chunks (1024)
PREW_R, PREW_C, PREW_G, PREW_COLS = 0, 1024, 2048, 3072
# per-step EW blob ([128, 4096]): W1e (1024) | W2 (2048) | W3 (1024)
EW_1, EW_2, EW_3, EW_COLS = 0, 1024, 3072, 4096
# per-step NGW blob: node W1 incl. glob part (4096) | nW2 (2048) | nW3 (1024)
#                    | gW1 (3072) | gW2 (2048) | gW3 (1024)
NGW_N1, NGW_N2, NGW_N3 = 0, 4096, 6144
NGW_G1, NGW_G2, NGW_G3 = 7168, 10240, 12288
NGW_COLS = 13312


def _chunked(w):
    """[K, M] weight -> [128, (K//128)*M] host layout matching device slicing:
    cols kh*M + m  hold  w[kh*128 + p, m]."""
    k, m = w.shape
    assert k % 128 == 0
    return np.ascontiguousarray(
        w.reshape(k // 128, 128, m).transpose(1, 0, 2).reshape(128, (k // 128) * m)
    ).astype(np.float32)


# ---------------------------------------------------------------------------
# device program
# ---------------------------------------------------------------------------


def _mm_chain(nc, psum, lhsT_slices, rhs_slices):
    n = len(lhsT_slices)
    for i, (lt, rh) in enumerate(zip(lhsT_slices, rhs_slices)):
        nc.tensor.matmul(psum, lt, rh, start=(i == 0), stop=(i == n - 1))


def build_program(steps=STEPS):
    nc = bacc.Bacc("TRN2", target_bir_lowering=False)

    # ---- DRAM tensors ----
    pos = nc.dram_tensor("pos", [3, GPC * N], F32, kind="ExternalInput")
    maska = nc.dram_tensor("maska", [N, N], F32, kind="ExternalInput")
    maskb = nc.dram_tensor("maskb", [N, N], F32, kind="ExternalInput")
    biasb = nc.dram_tensor("biasb", [128, BIAS_COLS], F32, kind="ExternalInput")
    pna = nc.dram_tensor("pna", [N, H], F32R, kind="ExternalInput")
    pnb = nc.dram_tensor("pnb", [N, H], F32R, kind="ExternalInput")
    pn2 = nc.dram_tensor("pn2", [128, 4 * H], F32R, kind="ExternalInput")
    pn3 = nc.dram_tensor("pn3", [128, 4 * L], F32R, kind="ExternalInput")
    pe2 = nc.dram_tensor("pe2", [128, 4 * H], F32R, kind="ExternalInput")
    pe3 = nc.dram_tensor("pe3", [128, 4 * L], F32R, kind="ExternalInput")
    pg1 = nc.dram_tensor("pg1", [128, 32 * H], F32R, kind="ExternalInput")
    pg2 = nc.dram_tensor("pg2", [128, 4 * H], F32R, kind="ExternalInput")
    pg3 = nc.dram_tensor("pg3", [128, 4 * L], F32R, kind="ExternalInput")
    prew = nc.dram_tensor("prew", [steps, 128, PREW_COLS], F32R, kind="ExternalInput")
    ew = nc.dram_tensor("ew", [steps, 128, EW_COLS], F32R, kind="ExternalInput")
    ngw = nc.dram_tensor("ngw", [steps, 128, NGW_COLS], F32R, kind="ExternalInput")
    c65d = nc.dram_tensor("c65", [N + 9, ET], F32R, kind="ExternalInput")
    eb1r = nc.dram_tensor("eb1r", [steps, H], F32R, kind="ExternalInput")

    node_out = nc.dram_tensor("node_out", [L, GPC * N], F32R, kind="ExternalOutput")
    edge_out = nc.dram_tensor("edge_out", [L, NE], F32R, kind="ExternalOutput")
    glob_out = nc.dram_tensor("glob_out", [L, GPC], F32R, kind="ExternalOutput")

    with tile.TileContext(nc) as tc, \
            nc.allow_low_precision(reason="float32r pipeline is intentional"):
        import contextlib
        ctx = contextlib.ExitStack()
        with ctx:
            persist = ctx.enter_context(tc.tile_pool(name="persist", bufs=1))
            steptmp = ctx.enter_context(tc.tile_pool(name="steptmp", bufs=1))
            wpre = ctx.enter_context(tc.tile_pool(name="wpre", bufs=1))
            wew = ctx.enter_context(tc.tile_pool(name="wew", bufs=1))
            wngw = ctx.enter_context(tc.tile_pool(name="wngw", bufs=1))
            einp = ctx.enter_context(tc.tile_pool(name="einp", bufs=2))
            emid = ctx.enter_context(tc.tile_pool(name="emid", bufs=2))
            eout_p = ctx.enter_context(tc.tile_pool(name="eoutp", bufs=2))
            init1 = ctx.enter_context(tc.tile_pool(name="init1", bufs=1))
            stream = ctx.enter_context(tc.tile_pool(name="stream", bufs=3))
            misc = ctx.enter_context(tc.tile_pool(name="misc", bufs=1))
            ps = ctx.enter_context(tc.tile_pool(name="ps", bufs=4, space="PSUM"))
            ps3p = ctx.enter_context(tc.tile_pool(name="ps3p", bufs=2, space="PSUM"))
            dram = ctx.enter_context(tc.tile_pool(name="dram", bufs=1, space="DRAM"))

            _psn = [0]

            def psum512(w=ET):
                _psn[0] += 1
                t = ps.tile([128, ET], F32, tag="ps", name=f"ps{_psn[0]}")
                return t[:, :w] if w != ET else t

            # ---- persistent state ----
            biast = persist.tile([128, BIAS_COLS], F32)
            nc.sync.dma_start(biast[:], biasb[:])

            def bvec(name, m):
                c, _ = BIAS_OFF[name]
                return biast[:, c + m:c + m + 1]

            nodeT = persist.tile([128, 2, GPC * N], F32R)   # node state [f, n]
            globT = persist.tile([128, 2, GPC], F32R)       # glob state [f, g]
            sentT = persist.tile([128, 2, GPC, N], F32R)
            recvT = persist.tile([128, 2, GPC, N], F32R)
            gnT = persist.tile([128, 2, GPC], F32R)
            geT = persist.tile([128, 2, GPC], F32R)
            rs = [persist.tile([128, 2, GPC, N], F32, name=f"rs{i}") for i in range(2)]
            cs = [persist.tile([128, 2, GPC, N], F32, name=f"cs{i}") for i in range(2)]
            c65t = persist.tile([N + 1, ET], F32R)
            nc.sync.dma_start(c65t[:], c65d[:N + 1, :])
            onesr = persist.tile([1, ET], F32R)
            nc.sync.dma_start(onesr[:], c65d[N:N + 1, :])


            edg = dram.tile([2, L, NE], F32R)               # edge state ping-pong

            def edg_ap(buf, t):
                return edg[buf].rearrange("(fh p) e -> p fh e", p=128)[
                    :, :, t * ET:(t + 1) * ET]

            # ================= distance matrix (plain f32) =================
            pT = init1.tile([3, GPC * N], F32, tag="pT")
            nc.sync.dma_start(pT[:], pos[:])
            m2pT = init1.tile([3, GPC * N], F32, tag="pT2")
            nc.vector.tensor_scalar_mul(m2pT[:], pT[:], -2.0)
            sq = init1.tile([3, GPC * N], F32, tag="pT3")
            nc.vector.tensor_mul(sq[:], pT[:], pT[:])
            ones3 = init1.tile([3, 1], F32, tag="ones3")
            nc.vector.memset(ones3[:], 1.0)
            ones1 = init1.tile([1, N], F32, tag="ones1")
            nc.vector.memset(ones1[:], 1.0)
            rps = psum512()[:1, :GPC * N]
            nc.tensor.matmul(rps, ones3[:], sq[:], start=True, stop=True)
            rr = init1.tile([1, GPC * N], F32, tag="rr")
            nc.scalar.activation(rr[:], rps, AF.Copy)

            Dt = persist.tile([N, GPC, N], F32)
            for g in range(GPC):
                d2 = psum512()[:N, :N]
                sl = slice(g * N, (g + 1) * N)
                nc.tensor.matmul(d2, pT[:, sl], m2pT[:, sl], start=True, stop=False)
                nc.tensor.matmul(d2, ones1[:], rr[:, sl], start=False, stop=False)
                nc.tensor.matmul(d2, rr[:, sl], ones1[:], start=False, stop=True)
                nc.vector.tensor_scalar_max(Dt[:, g, :], d2, 0.0)
                nc.scalar.activation(Dt[:, g, :], Dt[:, g, :], AF.Sqrt)

            # D to DRAM scratch, [g, i*64+j] layout
            dd = dram.tile([GPC, NN], F32)
            dd_dst = bass.AP(tensor=dd[:].tensor, offset=dd[:].offset,
                             ap=[[N, N], [NN, GPC], [1, N]])
            nc.sync.dma_start(dd_dst, Dt[:])

            # ================= posen -> node init =================
            mka = persist.tile([N, N], F32)
            nc.sync.dma_start(mka[:], maska[:])
            mkb = persist.tile([N, N], F32)
            nc.sync.dma_start(mkb[:], maskb[:])
            rhsA = init1.tile([N, GPC, N], F32R, tag="rhsA")
            rhsB = init1.tile([N, GPC, N], F32R, tag="rhsB")
            for g in range(GPC):
                nc.vector.tensor_mul(rhsA[:, g, :], Dt[:, g, :], mka[:])
                nc.vector.tensor_mul(rhsB[:, g, :], Dt[:, g, :], mkb[:])
            pnat = init1.tile([N, H], F32R, tag="pnat")
            nc.sync.dma_start(pnat[:], pna[:])
            pnbt = init1.tile([N, H], F32R, tag="pnbt")
            nc.sync.dma_start(pnbt[:], pnb[:])

            h1n = misc.tile([128, 4, GPC * N], F32R, tag="h1n")
            for m in range(4):
                p1 = psum512(GPC * N)
                for g in range(GPC):
                    osl = p1[:, g * N:(g + 1) * N]
                    nc.tensor.matmul(osl, pnat[:, m * 128:(m + 1) * 128],
                                     rhsA[:, g, :], start=True, stop=False)
                    nc.tensor.matmul(osl, pnbt[:, m * 128:(m + 1) * 128],
                                     rhsB[:, g, :], start=False, stop=True)
                nc.scalar.activation(h1n[:, m, :], p1, AF.Relu, bias=bvec("pn_b1", m))

            iew_n = wew.tile([128, EW_COLS], F32R, tag="ewt", name="iew_n")
            nc.sync.dma_start(iew_n[:, :4 * H], pn2[:])
            nc.sync.dma_start(iew_n[:, 4 * H:4 * H + 4 * L], pn3[:])
            pn2t = iew_n[:, :4 * H].rearrange("p (kh m) -> p kh m", kh=4)
            h2n = misc.tile([128, 4, GPC * N], F32R, tag="h2n")
            for m in range(4):
                p2 = psum512(GPC * N)
                _mm_chain(nc, p2,
                          [pn2t[:, kh, m * 128:(m + 1) * 128] for kh in range(4)],
                          [h1n[:, kh, :] for kh in range(4)])
                nc.scalar.activation(h2n[:, m, :], p2, AF.Relu, bias=bvec("pn_b2", m))
            pn3t = iew_n[:, 4 * H:4 * H + 4 * L].rearrange("p (kh m) -> p kh m", kh=4)
            for m2 in range(2):
                p3 = psum512(GPC * N)
                _mm_chain(nc, p3,
                          [pn3t[:, kh, m2 * 128:(m2 + 1) * 128] for kh in range(4)],
                          [h2n[:, kh, :] for kh in range(4)])
                nc.vector.tensor_scalar(nodeT[:, m2, :], p3, bvec("pn_b3", m2),
                                        None, OP.add)

            # ================= posee -> edge init =================
            iew_e = wew.tile([128, EW_COLS], F32R, tag="ewt", name="iew_e")
            nc.sync.dma_start(iew_e[:, :4 * H], pe2[:])
            nc.sync.dma_start(iew_e[:, 4 * H:4 * H + 4 * L], pe3[:])
            pe2t = iew_e[:, :4 * H].rearrange("p (kh m) -> p kh m", kh=4)
            pe3t = iew_e[:, 4 * H:4 * H + 4 * L].rearrange("p (kh m) -> p kh m", kh=4)
            nc.vector.memset(cs[0][:], 0.0)
            for t in range(NT):
                g, i0 = t // TPG, (t % TPG) * RT
                db = einp.tile([128, ET], F32, tag="ein", name="db")
                src = bass.AP(tensor=dd[:].tensor, offset=dd[:].offset + t * ET,
                              ap=[[0, 128], [1, ET]])
                nc.sync.dma_start(db[:], src)
                h1 = emid.tile([128, 4, ET], F32R, tag="h1")
                for m in range(4):
                    nc.scalar.activation(h1[:, m, :], db[:], AF.Relu,
                                         bias=bvec("pe_b1", m),
                                         scale=bvec("pe_w1", m))
                h2 = emid.tile([128, 4, ET], F32R, tag="h2")
                for m in range(4):
                    p2 = psum512()
                    _mm_chain(nc, p2,
                              [pe2t[:, kh, m * 128:(m + 1) * 128] for kh in range(4)],
                              [h1[:, kh, :] for kh in range(4)])
                    nc.scalar.activation(h2[:, m, :], p2, AF.Relu,
                                         bias=bvec("pe_b2", m))
                eo = eout_p.tile([128, 2, ET], F32R, tag="eout")
                for m2 in range(2):
                    p3 = psum512()
                    _mm_chain(nc, p3,
                              [pe3t[:, kh, m2 * 128:(m2 + 1) * 128] for kh in range(4)],
                              [h2[:, kh, :] for kh in range(4)])
                    nc.vector.tensor_scalar(eo[:, m2, :], p3, bvec("pe_b3", m2),
                                            None, OP.add)
                nc.sync.dma_start(edg_ap(0, t), eo[:])
                eov = eo.rearrange("p fh (r j) -> p fh r j", r=RT)
                nc.vector.reduce_sum(rs[0][:, :, g, i0:i0 + RT], eov,
                                     axis=mybir.AxisListType.X)
                for r in range(RT):
                    nc.gpsimd.tensor_tensor(cs[0][:, :, g, :], cs[0][:, :, g, :],
                                            eov[:, :, r, :], OP.add)

            # ================= poseg -> glob init =================
            rhsG = init1.tile([128, 32, GPC], F32R, tag="rhsG")
            for g in range(GPC):
                srcg = bass.AP(tensor=dd[:].tensor, offset=dd[:].offset + g * NN,
                               ap=[[1, 128], [128, 32]])
                nc.gpsimd.dma_start(rhsG[:, :, g], srcg)
            gps = [psum512(GPC) for _ in range(4)]
            for kc in range(32):
                pg1t = emid.tile([128, H], F32R, tag="h2", name="pg1t")
                nc.sync.dma_start(pg1t[:], pg1[:, kc * H:(kc + 1) * H])
                for m in range(4):
                    nc.tensor.matmul(gps[m], pg1t[:, m * 128:(m + 1) * 128],
                                     rhsG[:, kc, :], start=(kc == 0), stop=(kc == 31))
            h1g = misc.tile([128, 4, GPC], F32R, tag="h1g")
            for m in range(4):
                nc.scalar.activation(h1g[:, m, :], gps[m], AF.Relu,
                                     bias=bvec("pg_b1", m))
            iew_g = wew.tile([128, EW_COLS], F32R, tag="ewt", name="iew_g")
            nc.sync.dma_start(iew_g[:, :4 * H], pg2[:])
            nc.sync.dma_start(iew_g[:, 4 * H:4 * H + 4 * L], pg3[:])
            pg2t = iew_g[:, :4 * H].rearrange("p (kh m) -> p kh m", kh=4)
            h2g = misc.tile([128, 4, GPC], F32R, tag="h2g")
            for m in range(4):
                p2 = psum512(GPC)
                _mm_chain(nc, p2,
                          [pg2t[:, kh, m * 128:(m + 1) * 128] for kh in range(4)],
                          [h1g[:, kh, :] for kh in range(4)])
                nc.scalar.activation(h2g[:, m, :], p2, AF.Relu, bias=bvec("pg_b2", m))
            pg3t = iew_g[:, 4 * H:4 * H + 4 * L].rearrange("p (kh m) -> p kh m", kh=4)
            for m2 in range(2):
                p3 = psum512(GPC)
                _mm_chain(nc, p3,
                          [pg3t[:, kh, m2 * 128:(m2 + 1) * 128] for kh in range(4)],
                          [h2g[:, kh, :] for kh in range(4)])
                nc.vector.tensor_scalar(globT[:, m2, :], p3, bvec("pg_b3", m2),
                                        None, OP.add)

            # ================= message-passing steps =================
            # Per-graph software pipeline: after graph g's 8 edge tiles,
            # immediately run its node/glob MLPs and the *next* step's
            # per-graph precompute, so the PE never drains at step
            # boundaries (tiles of the following graphs overlap them).

            def precompute(s, g, prewt, eb1t, srcT, dstNM, gtmps, asmB):
                """Emit per-graph L1-fold precompute for step s."""
                # srcT[:, :, g*N:(g+1)*N] : (node @ W1r)^T feature-major
                for m in range(4):
                    p1 = psum512(N)
                    _mm_chain(nc, p1,
                              [prewt[:, PREW_R + kh * H + m * 128:
                                     PREW_R + kh * H + m * 128 + 128]
                               for kh in range(2)],
                              [nodeT[:, kh, g * N:(g + 1) * N] for kh in range(2)])
                    nc.scalar.activation(srcT[:, m, g * N:(g + 1) * N], p1, AF.Copy)
                # dstNM rows for graph g: node @ W1c node-major
                pd = psum512()[:N, :]
                _mm_chain(nc, pd,
                          [nodeT[:, kh, g * N:(g + 1) * N] for kh in range(2)],
                          [prewt[:, PREW_C + kh * H:PREW_C + (kh + 1) * H]
                           for kh in range(2)])
                nc.scalar.activation(
                    dstNM[(g % 2) * N:(g % 2) * N + N, g // 2, :], pd, AF.Copy)
                # glob+bias row for graph g
                pgr = psum512()[:1, :]
                nc.tensor.matmul(pgr, globT[:, 0, g:g + 1],
                                 prewt[:, PREW_G:PREW_G + H],
                                 start=True, stop=False)
                nc.tensor.matmul(pgr, globT[:, 1, g:g + 1],
                                 prewt[:, PREW_G + H:PREW_G + 2 * H],
                                 start=False, stop=False)
                nc.tensor.matmul(pgr, onesr[:, :1], eb1t[:], start=False, stop=True)
                nc.scalar.activation(gtmps[0:1, g * H:(g + 1) * H], pgr, AF.Copy)
                ab = asmB[g]
                nc.sync.dma_start(
                    ab[:N, :], dstNM[(g % 2) * N:(g % 2) * N + N, g // 2, :])
                nc.sync.dma_start(ab[N:N + 1, :], gtmps[0:1, g * H:(g + 1) * H])

            prewt = wpre.tile([128, PREW_COLS], F32R, tag="prew", name="prew0")
            nc.sync.dma_start(prewt[:], prew[0])
            eb1t = steptmp.tile([1, H], F32R, tag="eb1t", name="eb1t0")
            nc.sync.dma_start(eb1t[:], eb1r[0:1, :])
            srcT = steptmp.tile([128, 4, GPC * N], F32, tag="srcT", name="srcT0")
            dstNM = steptmp.tile([128, 2, H], F32R, tag="dstNM", name="dstNM0")
            gtmps = steptmp.tile([1, GPC * H], F32R, tag="gtmps", name="gtmps0")
            asmB = [steptmp.tile([N + 1, H], F32R, tag=f"asmB{g}",
                                 name=f"asmB0_{g}") for g in range(GPC)]
            for g in range(GPC):
                precompute(0, g, prewt, eb1t, srcT, dstNM, gtmps, asmB)

            for s in range(steps):
                cur, nxt = s % 2, (s + 1) % 2
                last = s == steps - 1
                ewt = wew.tile([128, EW_COLS], F32R, tag="ewt")
                nc.sync.dma_start(ewt[:], ew[s])
                ngwt = wngw.tile([128, NGW_COLS], F32R, tag="ngw")
                nc.sync.dma_start(ngwt[:], ngw[s])
                if not last:
                    prewt = wpre.tile([128, PREW_COLS], F32R, tag="prew",
                                      name=f"prew{s + 1}")
                    nc.sync.dma_start(prewt[:], prew[s + 1])
                    eb1t = steptmp.tile([1, H], F32R, tag="eb1t",
                                        name=f"eb1t{s + 1}")
                    nc.sync.dma_start(eb1t[:], eb1r[s + 1:s + 2, :])
                    srcT_n = steptmp.tile([128, 4, GPC * N], F32, tag="srcT2"
                                          if s % 2 == 0 else "srcT",
                                          name=f"srcT{s + 1}")
                    dstNM_n = steptmp.tile([128, 2, H], F32R, tag="dstNM",
                                           name=f"dstNM{s + 1}")
                    gtmps_n = steptmp.tile([1, GPC * H], F32R, tag="gtmps",
                                           name=f"gtmps{s + 1}")
                    asmB_n = [steptmp.tile([N + 1, H], F32R, tag=f"asmB{g}",
                                           name=f"asmB{s + 1}_{g}")
                              for g in range(GPC)]
                rs_prev, rs_cur = rs[s % 2], rs[(s + 1) % 2]
                cs_prev, cs_cur = cs[s % 2], cs[(s + 1) % 2]

                for g in range(GPC):
                    nc.gpsimd.memset(cs_cur[:, :, g, :].bitcast(F32), 0.0)
                    for tt_ in range(TPG):
                        t = g * TPG + tt_
                        i0 = tt_ * RT
                        ein = einp.tile([128, 2, ET], F32R, tag="ein")
                        nc.sync.dma_start(ein[:], edg_ap(cur, t))
                        h1 = emid.tile([128, 4, ET], F32R, tag="h1")
                        for m in range(4):
                            p1 = psum512()
                            _mm_chain(nc, p1,
                                      [ewt[:, EW_1 + kh * H + m * 128:
                                           EW_1 + kh * H + m * 128 + 128]
                                       for kh in range(2)]
                                      + [asmB[g][:, m * 128:(m + 1) * 128]],
                                      [ein[:, kh, :] for kh in range(2)]
                                      + [c65t[:]])
                            pv = p1.rearrange("p (r j) -> p r j", r=RT)
                            nc.vector.tensor_tensor(
                                pv, pv,
                                srcT[:, m, g * N + i0:g * N + i0 + RT]
                                .unsqueeze(2).to_broadcast([128, RT, N]), OP.add)
                            nc.scalar.activation(h1[:, m, :], p1, AF.Relu)
                        h2 = emid.tile([128, 4, ET], F32R, tag="h2")
                        for m in range(4):
                            p2 = psum512()
                            _mm_chain(nc, p2,
                                      [ewt[:, EW_2 + kh * H + m * 128:
                                           EW_2 + kh * H + m * 128 + 128]
                                       for kh in range(4)],
                                      [h1[:, kh, :] for kh in range(4)])
                            nc.scalar.activation(h2[:, m, :], p2, AF.Relu,
                                                 bias=bvec(f"s{s}_e_b2", m))
                        p3 = ps3p.tile([128, 2 * ET], F32, tag="ps3",
                                       name=f"p3_{s}_{t}")
                        for m2 in range(2):
                            _mm_chain(nc, p3[:, m2 * ET:(m2 + 1) * ET],
                                      [ewt[:, EW_3 + kh * L + m2 * 128:
                                           EW_3 + kh * L + m2 * 128 + 128]
                                       for kh in range(4)],
                                      [h2[:, kh, :] for kh in range(4)])
                        dg = bass.AP(tensor=p3[:].tensor,
                                     offset=p3[:].offset + i0,
                                     ap=[[p3[:].ap[0][0], 128], [ET, 2],
                                         [N + 1, RT]])
                        nc.vector.memset(dg, 0.0)
                        p3v = p3.rearrange("p (fh e) -> p fh e", fh=2)
                        eo = eout_p.tile([128, 2, ET], F32R, tag="eout")
                        nc.vector.tensor_tensor(eo[:], p3v, ein[:], OP.add)
                        eov = eo.rearrange("p fh (r j) -> p fh r j", r=RT)
                        nc.vector.reduce_sum(rs_cur[:, :, g, i0:i0 + RT], eov,
                                             axis=mybir.AxisListType.X)
                        for r in range(RT):
                            nc.gpsimd.tensor_tensor(cs_cur[:, :, g, :],
                                                    cs_cur[:, :, g, :],
                                                    eov[:, :, r, :], OP.add)
                        if last:
                            for m2 in range(2):
                                nc.vector.tensor_scalar(
                                    eo[:, m2, :], eo[:, m2, :],
                                    bvec("e_b3cum", m2), None, OP.add)
                            nc.sync.dma_start(
                                edge_out.rearrange("(fh p) e -> p fh e", p=128)
                                [:, :, t * ET:(t + 1) * ET], eo[:])
                        else:
                            nc.sync.dma_start(edg_ap(nxt, t), eo[:])

                    # ---- per-graph node/glob phase ----
                    gs = slice(g * N, (g + 1) * N)
                    nc.vector.tensor_tensor(sentT[:, :, g, :], rs_cur[:, :, g, :],
                                            rs_prev[:, :, g, :], OP.subtract)
                    nc.vector.tensor_tensor(recvT[:, :, g, :], cs_cur[:, :, g, :],
                                            cs_prev[:, :, g, :], OP.subtract)
                    nc.vector.reduce_sum(geT[:, :, g:g + 1],
                                         sentT[:, :, g:g + 1, :],
                                         axis=mybir.AxisListType.X)
                    # node-MLP glob term (N=4; only column g is fresh/used)
                    gwn = misc.tile([128, 4, GPC], F32, tag="gwn",
                                    name=f"gwn{s}_{g}")
                    for m in range(4):
                        pg_ = psum512(GPC)
                        _mm_chain(nc, pg_,
                                  [ngwt[:, NGW_N1 + 3072 + kh * H + m * 128:
                                        NGW_N1 + 3072 + kh * H + m * 128 + 128]
                                   for kh in range(2)],
                                  [globT[:, kh, :] for kh in range(2)])
                        nc.scalar.activation(gwn[:, m, :], pg_, AF.Copy)
                    # node MLP (N=64)
                    rhs_n = [nodeT[:, 0, gs], nodeT[:, 1, gs],
                             sentT[:, 0, g, :], sentT[:, 1, g, :],
                             recvT[:, 0, g, :], recvT[:, 1, g, :]]
                    h1nd = misc.tile([128, 4, N], F32R, tag="h1n",
                                     name=f"h1n{s}_{g}")
                    for m in range(4):
                        p1 = psum512(N)
                        _mm_chain(nc, p1,
                                  [ngwt[:, NGW_N1 + kh * H + m * 128:
                                        NGW_N1 + kh * H + m * 128 + 128]
                                   for kh in range(6)], rhs_n)
                        nc.vector.tensor_tensor(
                            p1, p1,
                            gwn[:, m, g:g + 1].to_broadcast([128, N]), OP.add)
                        nc.scalar.activation(h1nd[:, m, :], p1, AF.Relu,
                                             bias=bvec(f"s{s}_n_b1", m))
                    h2nd = misc.tile([128, 4, N], F32R, tag="h2n",
                                     name=f"h2n{s}_{g}")
                    for m in range(4):
                        p2 = psum512(N)
                        _mm_chain(nc, p2,
                                  [ngwt[:, NGW_N2 + kh * H + m * 128:
                                        NGW_N2 + kh * H + m * 128 + 128]
                                   for kh in range(4)],
                                  [h1nd[:, kh, :] for kh in range(4)])
                        nc.scalar.activation(h2nd[:, m, :], p2, AF.Relu,
                                             bias=bvec(f"s{s}_n_b2", m))
                    n1t = misc.tile([128, 2, N], F32, tag="n1t",
                                    name=f"n1t{s}_{g}")
                    for m2 in range(2):
                        p3n = psum512(N)
                        _mm_chain(nc, p3n,
                                  [ngwt[:, NGW_N3 + kh * L + m2 * 128:
                                        NGW_N3 + kh * L + m2 * 128 + 128]
                                   for kh in range(4)],
                                  [h2nd[:, kh, :] for kh in range(4)])
                        nc.vector.tensor_scalar(n1t[:, m2, :], p3n,
                                                bvec(f"s{s}_n_b3", m2), None,
                                                OP.add)
                    nc.vector.reduce_sum(
                        gnT[:, :, g:g + 1],
                        n1t[:].unsqueeze(2),
                        axis=mybir.AxisListType.X)
                    # glob MLP (N=1)
                    rhs_g = [globT[:, 0, :], globT[:, 1, :],
                             gnT[:, 0, :], gnT[:, 1, :],
                             geT[:, 0, :], geT[:, 1, :]]
                    h1gd = misc.tile([128, 4, GPC], F32R, tag="h1g",
                                     name=f"h1g{s}_{g}")
                    for m in range(4):
                        p1 = psum512(GPC)
                        _mm_chain(nc, p1,
                                  [ngwt[:, NGW_G1 + kh * H + m * 128:
                                        NGW_G1 + kh * H + m * 128 + 128]
                                   for kh in range(6)], rhs_g)
                        nc.scalar.activation(h1gd[:, m, :], p1, AF.Relu,
                                             bias=bvec(f"s{s}_g_b1", m))
                    h2gd = misc.tile([128, 4, GPC], F32R, tag="h2g",
                                     name=f"h2g{s}_{g}")
                    for m in range(4):
                        p2 = psum512(GPC)
                        _mm_chain(nc, p2,
                                  [ngwt[:, NGW_G2 + kh * H + m * 128:
                                        NGW_G2 + kh * H + m * 128 + 128]
                                   for kh in range(4)],
                                  [h1gd[:, kh, :] for kh in range(4)])
                        nc.scalar.activation(h2gd[:, m, :], p2, AF.Relu,
                                             bias=bvec(f"s{s}_g_b2", m))
                    g1t = misc.tile([128, 2, GPC], F32, tag="g1t",
                                    name=f"g1t{s}_{g}")
                    for m2 in range(2):
                        p3g = psum512(GPC)
                        _mm_chain(nc, p3g,
                                  [ngwt[:, NGW_G3 + kh * L + m2 * 128:
                                        NGW_G3 + kh * L + m2 * 128 + 128]
                                   for kh in range(4)],
                                  [h2gd[:, kh, :] for kh in range(4)])
                        nc.vector.tensor_scalar(g1t[:, m2, :], p3g,
                                                bvec(f"s{s}_g_b3", m2), None,
                                                OP.add)
                    # residuals for graph g
                    nc.vector.tensor_tensor(nodeT[:, :, gs], nodeT[:, :, gs],
                                            n1t[:], OP.add)
                    nc.vector.tensor_tensor(
                        globT[:, :, g:g + 1], globT[:, :, g:g + 1],
                        g1t[:, :, g:g + 1], OP.add)
                    # next step's per-graph precompute
                    if not last:
                        precompute(s + 1, g, prewt, eb1t, srcT_n, dstNM_n,
                                   gtmps_n, asmB_n)

                if not last:
                    srcT, dstNM, gtmps, asmB = srcT_n, dstNM_n, gtmps_n, asmB_n

            # ---- outputs ----
            nc.sync.dma_start(
                node_out.rearrange("(fh p) n -> p fh n", p=128), nodeT[:])
            nc.sync.dma_start(
                glob_out.rearrange("(fh p) g -> p fh g", p=128), globT[:])

    nc.finalize()
    return nc


# ---------------------------------------------------------------------------
# host side
# ---------------------------------------------------------------------------


def _np(x):
    return np.asarray(x, dtype=np.float32)


def pack_weights(params, steps=STEPS):
    """Build the replicated weight arrays (everything except pos)."""
    pn = [( _np(l["w"]), _np(l["b"])) for l in params["posen"]]
    pe = [( _np(l["w"]), _np(l["b"])) for l in params["posee"]]
    pg = [( _np(l["w"]), _np(l["b"])) for l in params["poseg"]]
    gnn = [{k: [(_np(l["w"]), _np(l["b"])) for l in lp[k]]
            for k in ("edge", "node", "glob")} for lp in params["gnn"]]

    biasb = np.zeros((128, BIAS_COLS), np.float32)

    def put(name, v):
        c, w = BIAS_OFF[name]
        biasb[:, c:c + w] = v.reshape(w, 128).T

    put("pe_w1", pe[0][0][0])           # [512] first-layer weight row (in_dim=1)
    put("pe_b1", pe[0][1]); put("pe_b2", pe[1][1]); put("pe_b3", pe[2][1])
    put("pn_b1", pn[0][1]); put("pn_b2", pn[1][1]); put("pn_b3", pn[2][1])
    put("pg_b1", pg[0][1]); put("pg_b2", pg[1][1]); put("pg_b3", pg[2][1])

    # The edge-MLP output bias b3 is never added on-device (the e1 psum and
    # the stored edge state omit it).  All of its downstream effects are
    # folded into other biases here:
    #   - next step's edge-L1 bias gets W1e^T @ b3cum        (via eb1r rows)
    #   - node-L1 bias gets (Wn_s + Wn_r)^T @ (63*b3)        (sent/recv)
    #   - glob-L1 bias gets Wg_e^T @ (4032*b3)               (ge)
    #   - the final edge output adds b3cum explicitly         (e_b3cum)
    b3cum = np.zeros(L, np.float32)
    eb1r = np.zeros((steps, H), np.float32)
    for s in range(steps):
        e, n_, g_ = gnn[s]["edge"], gnn[s]["node"], gnn[s]["glob"]
        b3 = e[2][1]
        eb1r[s] = e[0][1] + b3cum @ e[0][0][:L]
        put(f"s{s}_e_b2", e[1][1])
        wn1, wg1 = n_[0][0], g_[0][0]
        put(f"s{s}_n_b1",
            n_[0][1] + (N - 1) * (b3 @ wn1[L:2 * L] + b3 @ wn1[2 * L:3 * L]))
        put(f"s{s}_n_b2", n_[1][1]); put(f"s{s}_n_b3", n_[2][1])
        put(f"s{s}_g_b1", g_[0][1] + N * (N - 1) * (b3 @ wg1[2 * L:]))
        put(f"s{s}_g_b2", g_[1][1]); put(f"s{s}_g_b3", g_[2][1])
        b3cum = b3cum + b3
    put("e_b3cum", b3cum)

    # L1 fold constants: col-selector rows + ones row (rows 65..72 unused)
    c65 = np.zeros((N + 9, ET), np.float32)
    for j in range(N):
        c65[j, :] = (np.arange(ET) % N == j)
    c65[N, :] = 1.0

    # posen masked weights
    w1n = pn[0][0]                       # [63, 512]
    wa = np.zeros((N, H), np.float32); wa[:N - 1] = w1n
    wb = np.zeros((N, H), np.float32); wb[1:] = w1n[:N - 1]
    maska = np.triu(np.ones((N, N), np.float32), 1)    # [j, i]: j < i
    maskb = np.tril(np.ones((N, N), np.float32), -1)   # [j, i]: j > i

    # poseg padded first layer [4096, 512]
    w1g = pg[0][0]
    wpad = np.zeros((NN, H), np.float32)
    idx_i = np.repeat(np.arange(N), N)
    idx_j = np.tile(np.arange(N), N)
    off = idx_i * (N - 1) + idx_j - (idx_j > idx_i)
    m = idx_i != idx_j
    wpad[np.arange(NN)[m]] = w1g[off[m]]

    prew = np.zeros((steps, 128, PREW_COLS), np.float32)
    ew = np.zeros((steps, 128, EW_COLS), np.float32)
    ngw = np.zeros((steps, 128, NGW_COLS), np.float32)
    for s in range(steps):
        e, n_, g_ = gnn[s]["edge"], gnn[s]["node"], gnn[s]["glob"]
        w1 = e[0][0]
        prew[s, :, PREW_R:PREW_R + 1024] = _chunked(w1[L:2 * L])
        prew[s, :, PREW_C:PREW_C + 1024] = _chunked(w1[2 * L:3 * L])
        prew[s, :, PREW_G:PREW_G + 1024] = _chunked(w1[3 * L:4 * L])
        ew[s, :, EW_1:EW_1 + 1024] = _chunked(w1[:L])
        ew[s, :, EW_2:EW_2 + 2048] = _chunked(e[1][0])
        ew[s, :, EW_3:EW_3 + 1024] = _chunked(e[2][0])
        wn1 = n_[0][0]
        ngw[s, :, NGW_N1:NGW_N1 + 3072] = _chunked(wn1[:3 * L])      # node|sent|recv
        ngw[s, :, NGW_N1 + 3072:NGW_N1 + 4096] = _chunked(wn1[3 * L:])  # glob part
        ngw[s, :, NGW_N2:NGW_N2 + 2048] = _chunked(n_[1][0])
        ngw[s, :, NGW_N3:NGW_N3 + 1024] = _chunked(n_[2][0])
        ngw[s, :, NGW_G1:NGW_G1 + 3072] = _chunked(g_[0][0])
        ngw[s, :, NGW_G2:NGW_G2 + 2048] = _chunked(g_[1][0])
        ngw[s, :, NGW_G3:NGW_G3 + 1024] = _chunked(g_[2][0])

    return {
        "maska": maska, "maskb": maskb, "biasb": biasb,
        "c65": c65, "eb1r": eb1r,
        "pna": wa, "pnb": wb,
        "pn2": _chunked(pn[1][0]), "pn3": _chunked(pn[2][0]),
        "pe2": _chunked(pe[1][0]), "pe3": _chunked(pe[2][0]),
        "pg1": _chunked(wpad), "pg2": _chunked(pg[1][0]),
        "pg3": _chunked(pg[2][0]),
        "prew": prew, "ew": ew, "ngw": ngw,
    }


_PROGRAM_CACHE = {}


TRACE = False          # set by test harness to capture a profile
LAST_RESULT = None


def kernel(conforms, params):
    global LAST_RESULT
    conforms = _np(conforms)
    wmap = pack_weights(params, STEPS)

    if STEPS not in _PROGRAM_CACHE:
        _PROGRAM_CACHE[STEPS] = build_program(STEPS)
    nc = _PROGRAM_CACHE[STEPS]

    in_maps = []
    for c in range(NCORES):
        posc = conforms[c * GPC:(c + 1) * GPC].reshape(GPC * N, 3).T
        in_maps.append({"pos": np.ascontiguousarray(posc), **wmap})

    res = run_bass_kernel_spmd(nc, in_maps, core_ids=list(range(NCORES)),
                               trace=TRACE)
    LAST_RESULT = res

    node = np.empty((B * N, L), np.float32)
    edge = np.empty((B * N * (N - 1), L), np.float32)
    glob = np.empty((B, L), np.float32)
    offd = ~np.eye(N, dtype=bool)
    EPG = N * (N - 1)
    for c in range(NCORES):
        r = res.results[c]
        node[c * GPC * N:(c + 1) * GPC * N] = r["node_out"].T
        glob[c * GPC:(c + 1) * GPC] = r["glob_out"].T
        ed = r["edge_out"].reshape(L, GPC, N, N).transpose(1, 2, 3, 0)
        for g in range(GPC):
            gi = c * GPC + g
            edge[gi * EPG:(gi + 1) * EPG] = ed[g][offd]
    return node, edge, glob


# revision 25
# speedup vs baseline: 1.1958x; 1.1958x over previous
"""Trainium2 Bass kernel for DMCG GNN message passing (B=32 fully-connected
graphs, N=64 nodes, 12 steps).

Strategy: data-parallel over graphs, 4 graphs per NeuronCore.  All state is
kept feature-major (features on SBUF partitions) so every MLP is a plain
chunked matmul, and the edge set is padded to the full 64x64 grid per graph
so gathers (node[ROWS]/node[COLS]) and scatter-sums (segment_sum over
src/dst/graph) become broadcast adds and axis reductions over a regular
[rows, cols] view.  Diagonal (self-loop) lanes are computed but excluded
exactly: triangular masks for the posen input, zero rows in the padded poseg
weight, and an explicit diagonal zeroing of each e1 tile before the
reductions / residual.

Matmuls run as float32r (full PE rate); the tiny distance-matrix matmuls run
as plain float32 for precision (sqrt near zero amplifies rounding).
"""

import numpy as np

import concourse.bass as bass
import concourse.mybir as mybir
import concourse.tile as tile
from concourse import bacc
from concourse.bass_utils import run_bass_kernel_spmd

# ---- problem sizes (hardcoded; must match the grader's reference) ----
B, N, L, H = 32, 64, 256, 512
STEPS = 12
NCORES = 8
GPC = B // NCORES          # graphs per core
NN = N * N                 # padded edges per graph (incl. diagonal)
NE = GPC * NN              # padded edges per core
ET = 512                   # edges per tile
RT = ET // N               # grid rows per tile
NT = NE // ET              # tiles per core
TPG = NN // ET             # tiles per graph

F32R = mybir.dt.float32r
F32 = mybir.dt.float32
AF = mybir.ActivationFunctionType
OP = mybir.AluOpType

# ---------------------------------------------------------------------------
# bias/scale blob layout (shared between host packing and device slicing)
# ---------------------------------------------------------------------------


def _bias_layout():
    names = [("pe_w1", H), ("pe_b1", H), ("pe_b2", H), ("pe_b3", L),
             ("pn_b1", H), ("pn_b2", H), ("pn_b3", L),
             ("pg_b1", H), ("pg_b2", H), ("pg_b3", L), ("e_b3cum", L)]
    for s in range(STEPS):
        for p, dims in (("e", (H, H, L)), ("n", (H, H, L)), ("g", (H, H, L))):
            for i, d in enumerate(dims):
                names.append((f"s{s}_{p}_b{i + 1}", d))
    off, c = {}, 0
    for nm, d in names:
        off[nm] = (c, d // 128)
        c += d // 128
    return off, c


BIAS_OFF, BIAS_COLS = _bias_layout()

# column layout inside the per-step PREW blob ([128, 3072]):
#   W1r chunks (1024) | W1c chunks (1024) | W1g chunks (1024)
PREW_R, PREW_C, PREW_G, PREW_COLS = 0, 1024, 2048, 3072
# per-step EW blob ([128, 4096]): W1e (1024) | W2 (2048) | W3 (1024)
EW_1, EW_2, EW_3, EW_COLS = 0, 1024, 3072, 4096
# per-step NGW blob: node W1 incl. glob part (4096) | nW2 (2048) | nW3 (1024)
#                    | gW1 (3072) | gW2 (2048) | gW3 (1024)
NGW_N1, NGW_N2, NGW_N3 = 0, 4096, 6144
NGW_G1, NGW_G2, NGW_G3 = 7168, 10240, 12288
NGW_COLS = 13312


def _chunked(w):
    """[K, M] weight -> [128, (K//128)*M] host layout matching device slicing:
    cols kh*M + m  hold  w[kh*128 + p, m]."""
    k, m = w.shape
    assert k % 128 == 0
    return np.ascontiguousarray(
        w.reshape(k // 128, 128, m).transpose(1, 0, 2).reshape(128, (k // 128) * m)
    ).astype(np.float32)


# ---------------------------------------------------------------------------
# device program
# ---------------------------------------------------------------------------


def _mm_chain(nc, psum, lhsT_slices, rhs_slices):
    n = len(lhsT_slices)
    for i, (lt, rh) in enumerate(zip(lhsT_slices, rhs_slices)):
        nc.tensor.matmul(psum, lt, rh, start=(i == 0), stop=(i == n - 1))


def build_program(steps=STEPS):
    nc = bacc.Bacc("TRN2", target_bir_lowering=False)

    # ---- DRAM tensors ----
    pos = nc.dram_tensor("pos", [3, GPC * N], F32, kind="ExternalInput")
    maska = nc.dram_tensor("maska", [N, N], F32, kind="ExternalInput")
    maskb = nc.dram_tensor("maskb", [N, N], F32, kind="ExternalInput")
    biasb = nc.dram_tensor("biasb", [128, BIAS_COLS], F32, kind="ExternalInput")
    pna = nc.dram_tensor("pna", [N, H], F32R, kind="ExternalInput")
    pnb = nc.dram_tensor("pnb", [N, H], F32R, kind="ExternalInput")
    pn2 = nc.dram_tensor("pn2", [128, 4 * H], F32R, kind="ExternalInput")
    pn3 = nc.dram_tensor("pn3", [128, 4 * L], F32R, kind="ExternalInput")
    pe2 = nc.dram_tensor("pe2", [128, 4 * H], F32R, kind="ExternalInput")
    pe3 = nc.dram_tensor("pe3", [128, 4 * L], F32R, kind="ExternalInput")
    pg1 = nc.dram_tensor("pg1", [128, 32 * H], F32R, kind="ExternalInput")
    pg2 = nc.dram_tensor("pg2", [128, 4 * H], F32R, kind="ExternalInput")
    pg3 = nc.dram_tensor("pg3", [128, 4 * L], F32R, kind="ExternalInput")
    prew = nc.dram_tensor("prew", [steps, 128, PREW_COLS], F32R, kind="ExternalInput")
    ew = nc.dram_tensor("ew", [steps, 128, EW_COLS], F32R, kind="ExternalInput")
    ngw = nc.dram_tensor("ngw", [steps, 128, NGW_COLS], F32R, kind="ExternalInput")
    c65d = nc.dram_tensor("c65", [N + 9, ET], F32R, kind="ExternalInput")
    eb1r = nc.dram_tensor("eb1r", [steps, H], F32R, kind="ExternalInput")

    node_out = nc.dram_tensor("node_out", [L, GPC * N], F32R, kind="ExternalOutput")
    edge_out = nc.dram_tensor("edge_out", [L, NE], F32R, kind="ExternalOutput")
    glob_out = nc.dram_tensor("glob_out", [L, GPC], F32R, kind="ExternalOutput")

    with tile.TileContext(nc) as tc, \
            nc.allow_low_precision(reason="float32r pipeline is intentional"):
        import contextlib
        ctx = contextlib.ExitStack()
        with ctx:
            persist = ctx.enter_context(tc.tile_pool(name="persist", bufs=1))
            steptmp = ctx.enter_context(tc.tile_pool(name="steptmp", bufs=1))
            wpre = ctx.enter_context(tc.tile_pool(name="wpre", bufs=1))
            wew = ctx.enter_context(tc.tile_pool(name="wew", bufs=1))
            wngw = ctx.enter_context(tc.tile_pool(name="wngw", bufs=1))
            einp = ctx.enter_context(tc.tile_pool(name="einp", bufs=2))
            emid = ctx.enter_context(tc.tile_pool(name="emid", bufs=2))
            eout_p = ctx.enter_context(tc.tile_pool(name="eoutp", bufs=2))
            init1 = ctx.enter_context(tc.tile_pool(name="init1", bufs=1))
            stream = ctx.enter_context(tc.tile_pool(name="stream", bufs=3))
            misc = ctx.enter_context(tc.tile_pool(name="misc", bufs=1))
            ps = ctx.enter_context(tc.tile_pool(name="ps", bufs=8, space="PSUM"))
            dram = ctx.enter_context(tc.tile_pool(name="dram", bufs=1, space="DRAM"))

            _psn = [0]

            def psum512(w=ET):
                _psn[0] += 1
                t = ps.tile([128, ET], F32, tag="ps", name=f"ps{_psn[0]}")
                return t[:, :w] if w != ET else t

            # ---- persistent state ----
            biast = persist.tile([128, BIAS_COLS], F32)
            nc.sync.dma_start(biast[:], biasb[:])

            def bvec(name, m):
                c, _ = BIAS_OFF[name]
                return biast[:, c + m:c + m + 1]

            nodeT = persist.tile([128, 2, GPC * N], F32R)   # node state [f, n]
            globT = persist.tile([128, 2, GPC], F32R)       # glob state [f, g]
            sentT = persist.tile([128, 2, GPC, N], F32R)
            recvT = persist.tile([128, 2, GPC, N], F32R)
            gnT = persist.tile([128, 2, GPC], F32R)
            geT = persist.tile([128, 2, GPC], F32R)
            rs = [persist.tile([128, 2, GPC, N], F32, name=f"rs{i}") for i in range(2)]
            cs = [persist.tile([128, 2, GPC, N], F32, name=f"cs{i}") for i in range(2)]
            c65t = persist.tile([N + 1, ET], F32R)
            nc.sync.dma_start(c65t[:], c65d[:N + 1, :])
            onesr = persist.tile([1, ET], F32R)
            nc.sync.dma_start(onesr[:], c65d[N:N + 1, :])


            edg = dram.tile([2, L, NE], F32R)               # edge state ping-pong

            def edg_ap(buf, t):
                return edg[buf].rearrange("(fh p) e -> p fh e", p=128)[
                    :, :, t * ET:(t + 1) * ET]

            # ================= distance matrix (plain f32) =================
            pT = init1.tile([3, GPC * N], F32, tag="pT")
            nc.sync.dma_start(pT[:], pos[:])
            m2pT = init1.tile([3, GPC * N], F32, tag="pT2")
            nc.vector.tensor_scalar_mul(m2pT[:], pT[:], -2.0)
            sq = init1.tile([3, GPC * N], F32, tag="pT3")
            nc.vector.tensor_mul(sq[:], pT[:], pT[:])
            ones3 = init1.tile([3, 1], F32, tag="ones3")
            nc.vector.memset(ones3[:], 1.0)
            ones1 = init1.tile([1, N], F32, tag="ones1")
            nc.vector.memset(ones1[:], 1.0)
            rps = psum512()[:1, :GPC * N]
            nc.tensor.matmul(rps, ones3[:], sq[:], start=True, stop=True)
            rr = init1.tile([1, GPC * N], F32, tag="rr")
            nc.scalar.activation(rr[:], rps, AF.Copy)

            Dt = persist.tile([N, GPC, N], F32)
            for g in range(GPC):
                d2 = psum512()[:N, :N]
                sl = slice(g * N, (g + 1) * N)
                nc.tensor.matmul(d2, pT[:, sl], m2pT[:, sl], start=True, stop=False)
                nc.tensor.matmul(d2, ones1[:], rr[:, sl], start=False, stop=False)
                nc.tensor.matmul(d2, rr[:, sl], ones1[:], start=False, stop=True)
                nc.vector.tensor_scalar_max(Dt[:, g, :], d2, 0.0)
                nc.scalar.activation(Dt[:, g, :], Dt[:, g, :], AF.Sqrt)

            # D to DRAM scratch, [g, i*64+j] layout
            dd = dram.tile([GPC, NN], F32)
            dd_dst = bass.AP(tensor=dd[:].tensor, offset=dd[:].offset,
                             ap=[[N, N], [NN, GPC], [1, N]])
            nc.sync.dma_start(dd_dst, Dt[:])

            # ================= posen -> node init =================
            mka = persist.tile([N, N], F32)
            nc.sync.dma_start(mka[:], maska[:])
            mkb = persist.tile([N, N], F32)
            nc.sync.dma_start(mkb[:], maskb[:])
            rhsA = init1.tile([N, GPC, N], F32R, tag="rhsA")
            rhsB = init1.tile([N, GPC, N], F32R, tag="rhsB")
            for g in range(GPC):
                nc.vector.tensor_mul(rhsA[:, g, :], Dt[:, g, :], mka[:])
                nc.vector.tensor_mul(rhsB[:, g, :], Dt[:, g, :], mkb[:])
            pnat = init1.tile([N, H], F32R, tag="pnat")
            nc.sync.dma_start(pnat[:], pna[:])
            pnbt = init1.tile([N, H], F32R, tag="pnbt")
            nc.sync.dma_start(pnbt[:], pnb[:])

            h1n = misc.tile([128, 4, GPC * N], F32R, tag="h1n")
            for m in range(4):
                p1 = psum512(GPC * N)
                for g in range(GPC):
                    osl = p1[:, g * N:(g + 1) * N]
                    nc.tensor.matmul(osl, pnat[:, m * 128:(m + 1) * 128],
                                     rhsA[:, g, :], start=True, stop=False)
                    nc.tensor.matmul(osl, pnbt[:, m * 128:(m + 1) * 128],
                                     rhsB[:, g, :], start=False, stop=True)
                nc.scalar.activation(h1n[:, m, :], p1, AF.Relu, bias=bvec("pn_b1", m))

            iew_n = wew.tile([128, EW_COLS], F32R, tag="ewt", name="iew_n")
            nc.sync.dma_start(iew_n[:, :4 * H], pn2[:])
            nc.sync.dma_start(iew_n[:, 4 * H:4 * H + 4 * L], pn3[:])
            pn2t = iew_n[:, :4 * H].rearrange("p (kh m) -> p kh m", kh=4)
            h2n = misc.tile([128, 4, GPC * N], F32R, tag="h2n")
            for m in range(4):
                p2 = psum512(GPC * N)
                _mm_chain(nc, p2,
                          [pn2t[:, kh, m * 128:(m + 1) * 128] for kh in range(4)],
                          [h1n[:, kh, :] for kh in range(4)])
                nc.scalar.activation(h2n[:, m, :], p2, AF.Relu, bias=bvec("pn_b2", m))
            pn3t = iew_n[:, 4 * H:4 * H + 4 * L].rearrange("p (kh m) -> p kh m", kh=4)
            for m2 in range(2):
                p3 = psum512(GPC * N)
                _mm_chain(nc, p3,
                          [pn3t[:, kh, m2 * 128:(m2 + 1) * 128] for kh in range(4)],
                          [h2n[:, kh, :] for kh in range(4)])
                nc.vector.tensor_scalar(nodeT[:, m2, :], p3, bvec("pn_b3", m2),
                                        None, OP.add)

            # ================= posee -> edge init =================
            iew_e = wew.tile([128, EW_COLS], F32R, tag="ewt", name="iew_e")
            nc.sync.dma_start(iew_e[:, :4 * H], pe2[:])
            nc.sync.dma_start(iew_e[:, 4 * H:4 * H + 4 * L], pe3[:])
            pe2t = iew_e[:, :4 * H].rearrange("p (kh m) -> p kh m", kh=4)
            pe3t = iew_e[:, 4 * H:4 * H + 4 * L].rearrange("p (kh m) -> p kh m", kh=4)
            nc.vector.memset(cs[0][:], 0.0)
            for t in range(NT):
                g, i0 = t // TPG, (t % TPG) * RT
                db = einp.tile([128, ET], F32, tag="ein", name="db")
                src = bass.AP(tensor=dd[:].tensor, offset=dd[:].offset + t * ET,
                              ap=[[0, 128], [1, ET]])
                nc.sync.dma_start(db[:], src)
                h1 = emid.tile([128, 4, ET], F32R, tag="h1")
                for m in range(4):
                    nc.scalar.activation(h1[:, m, :], db[:], AF.Relu,
                                         bias=bvec("pe_b1", m),
                                         scale=bvec("pe_w1", m))
                h2 = emid.tile([128, 4, ET], F32R, tag="h2")
                for m in range(4):
                    p2 = psum512()
                    _mm_chain(nc, p2,
                              [pe2t[:, kh, m * 128:(m + 1) * 128] for kh in range(4)],
                              [h1[:, kh, :] for kh in range(4)])
                    nc.scalar.activation(h2[:, m, :], p2, AF.Relu,
                                         bias=bvec("pe_b2", m))
                eo = eout_p.tile([128, 2, ET], F32R, tag="eout")
                for m2 in range(2):
                    p3 = psum512()
                    _mm_chain(nc, p3,
                              [pe3t[:, kh, m2 * 128:(m2 + 1) * 128] for kh in range(4)],
                              [h2[:, kh, :] for kh in range(4)])
                    nc.vector.tensor_scalar(eo[:, m2, :], p3, bvec("pe_b3", m2),
                                            None, OP.add)
                nc.sync.dma_start(edg_ap(0, t), eo[:])
                eov = eo.rearrange("p fh (r j) -> p fh r j", r=RT)
                nc.vector.reduce_sum(rs[0][:, :, g, i0:i0 + RT], eov,
                                     axis=mybir.AxisListType.X)
                for r in range(RT):
                    nc.gpsimd.tensor_tensor(cs[0][:, :, g, :], cs[0][:, :, g, :],
                                            eov[:, :, r, :], OP.add)

            # ================= poseg -> glob init =================
            rhsG = init1.tile([128, 32, GPC], F32R, tag="rhsG")
            for g in range(GPC):
                srcg = bass.AP(tensor=dd[:].tensor, offset=dd[:].offset + g * NN,
                               ap=[[1, 128], [128, 32]])
                nc.gpsimd.dma_start(rhsG[:, :, g], srcg)
            gps = [psum512(GPC) for _ in range(4)]
            for kc in range(32):
                pg1t = emid.tile([128, H], F32R, tag="h2", name="pg1t")
                nc.sync.dma_start(pg1t[:], pg1[:, kc * H:(kc + 1) * H])
                for m in range(4):
                    nc.tensor.matmul(gps[m], pg1t[:, m * 128:(m + 1) * 128],
                                     rhsG[:, kc, :], start=(kc == 0), stop=(kc == 31))
            h1g = misc.tile([128, 4, GPC], F32R, tag="h1g")
            for m in range(4):
                nc.scalar.activation(h1g[:, m, :], gps[m], AF.Relu,
                                     bias=bvec("pg_b1", m))
            iew_g = wew.tile([128, EW_COLS], F32R, tag="ewt", name="iew_g")
            nc.sync.dma_start(iew_g[:, :4 * H], pg2[:])
            nc.sync.dma_start(iew_g[:, 4 * H:4 * H + 4 * L], pg3[:])
            pg2t = iew_g[:, :4 * H].rearrange("p (kh m) -> p kh m", kh=4)
            h2g = misc.tile([128, 4, GPC], F32R, tag="h2g")
            for m in range(4):
                p2 = psum512(GPC)
                _mm_chain(nc, p2,
                          [pg2t[:, kh, m * 128:(m + 1) * 128] for kh in range(4)],
                          [h1g[:, kh, :] for kh in range(4)])
                nc.scalar.activation(h2g[:, m, :], p2, AF.Relu, bias=bvec("pg_b2", m))
            pg3t = iew_g[:, 4 * H:4 * H + 4 * L].rearrange("p (kh m) -> p kh m", kh=4)
            for m2 in range(2):
                p3 = psum512(GPC)
                _mm_chain(nc, p3,
                          [pg3t[:, kh, m2 * 128:(m2 + 1) * 128] for kh in range(4)],
                          [h2g[:, kh, :] for kh in range(4)])
                nc.vector.tensor_scalar(globT[:, m2, :], p3, bvec("pg_b3", m2),
                                        None, OP.add)

            # ================= message-passing steps =================
            for s in range(steps):
                cur, nxt = s % 2, (s + 1) % 2
                prewt = wpre.tile([128, PREW_COLS], F32R, tag="prew")
                nc.sync.dma_start(prewt[:], prew[s])
                ewt = wew.tile([128, EW_COLS], F32R, tag="ewt")
                nc.sync.dma_start(ewt[:], ew[s])
                ngwt = wngw.tile([128, NGW_COLS], F32R, tag="ngw")
                nc.sync.dma_start(ngwt[:], ngw[s])

                # --- precompute per-node/per-graph L1 terms ---
                # srcT: (node @ W1r)^T, feature-major, for the DVE row add
                srcT = steptmp.tile([128, 4, GPC * N], F32, tag="srcT")
                for m in range(4):
                    p1 = psum512(GPC * N)
                    _mm_chain(nc, p1,
                              [prewt[:, PREW_R + kh * H + m * 128:
                                     PREW_R + kh * H + m * 128 + 128]
                               for kh in range(2)],
                              [nodeT[:, kh, :] for kh in range(2)])
                    nc.scalar.activation(srcT[:, m, :], p1, AF.Copy)
                # dstNM: node @ W1c, node-major, feeds the K=65 asm chunk
                dstNM = steptmp.tile([128, 2, H], F32R, tag="dstNM")
                for ns in range(2):
                    pd = psum512()
                    _mm_chain(nc, pd,
                              [nodeT[:, kh, ns * 128:(ns + 1) * 128]
                               for kh in range(2)],
                              [prewt[:, PREW_C + kh * H:PREW_C + (kh + 1) * H]
                               for kh in range(2)])
                    nc.scalar.activation(dstNM[:, ns, :], pd, AF.Copy)
                # per-graph glob+bias row: glob @ W1g + e_b1_adj  [GPC, 512]
                eb1t = steptmp.tile([1, H], F32R, tag="eb1t")
                nc.sync.dma_start(eb1t[:], eb1r[s:s + 1, :])
                pgr = psum512()[:GPC, :]
                nc.tensor.matmul(pgr, globT[:, 0, :],
                                 prewt[:, PREW_G:PREW_G + H], start=True, stop=False)
                nc.tensor.matmul(pgr, globT[:, 1, :],
                                 prewt[:, PREW_G + H:PREW_G + 2 * H],
                                 start=False, stop=False)
                nc.tensor.matmul(pgr, onesr[:, :GPC], eb1t[:], start=False, stop=True)
                gbt = steptmp.tile([GPC, H], F32R, tag="gbt")
                nc.scalar.activation(gbt[:], pgr, AF.Copy)
                # assemble per-graph lhsT blocks [65, H]: dst rows + glob/bias row
                asmB = []
                for g in range(GPC):
                    ab = steptmp.tile([N + 1, H], F32R, tag=f"asmB{g}")
                    nc.sync.dma_start(
                        ab[:N, :], dstNM[(g % 2) * N:(g % 2) * N + N, g // 2, :])
                    nc.sync.dma_start(ab[N:N + 1, :], gbt[g:g + 1, :])
                    asmB.append(ab)
                rs_prev, rs_cur = rs[s % 2], rs[(s + 1) % 2]
                cs_prev, cs_cur = cs[s % 2], cs[(s + 1) % 2]
                nc.gpsimd.memset(cs_cur[:], 0.0)

                # --- edge tile loop ---
                for t in range(NT):
                    g, i0 = t // TPG, (t % TPG) * RT
                    ein = einp.tile([128, 2, ET], F32R, tag="ein")
                    nc.sync.dma_start(ein[:], edg_ap(cur, t))
                    h1 = emid.tile([128, 4, ET], F32R, tag="h1")
                    for m in range(4):
                        p1 = psum512()
                        _mm_chain(nc, p1,
                                  [ewt[:, EW_1 + kh * H + m * 128:
                                       EW_1 + kh * H + m * 128 + 128]
                                   for kh in range(2)]
                                  + [asmB[g][:, m * 128:(m + 1) * 128]],
                                  [ein[:, kh, :] for kh in range(2)] + [c65t[:]])
                        pv = p1.rearrange("p (r j) -> p r j", r=RT)
                        nc.vector.tensor_tensor(
                            pv, pv,
                            srcT[:, m, g * N + i0:g * N + i0 + RT]
                            .unsqueeze(2).to_broadcast([128, RT, N]), OP.add)
                        nc.scalar.activation(h1[:, m, :], p1, AF.Relu)
                    h2 = emid.tile([128, 4, ET], F32R, tag="h2")
                    for m in range(4):
                        p2 = psum512()
                        _mm_chain(nc, p2,
                                  [ewt[:, EW_2 + kh * H + m * 128:
                                       EW_2 + kh * H + m * 128 + 128]
                                   for kh in range(4)],
                                  [h1[:, kh, :] for kh in range(4)])
                        nc.scalar.activation(h2[:, m, :], p2, AF.Relu,
                                             bias=bvec(f"s{s}_e_b2", m))
                    eo = eout_p.tile([128, 2, ET], F32R, tag="eout")
                    for m2 in range(2):
                        p3 = psum512()
                        _mm_chain(nc, p3,
                                  [ewt[:, EW_3 + kh * L + m2 * 128:
                                       EW_3 + kh * L + m2 * 128 + 128]
                                   for kh in range(4)],
                                  [h2[:, kh, :] for kh in range(4)])
                        # zero the diagonal lanes present in this tile (psum)
                        dg = bass.AP(tensor=p3[:].tensor,
                                     offset=p3[:].offset + i0,
                                     ap=[[p3[:].ap[0][0], 128], [N + 1, RT]])
                        nc.vector.memset(dg, 0.0)
                        # residual edge update (e1 here lacks b3 — folded into
                        # the next step's L1 bias host-side; applied on the
                        # last step below)
                        nc.vector.tensor_tensor(eo[:, m2, :], p3, ein[:, m2, :],
                                                OP.add)
                    # row/col sums of the new edge state (gpsimd, SBUF);
                    # sent/recv are recovered as deltas next
                    eov = eo.rearrange("p fh (r j) -> p fh r j", r=RT)
                    nc.vector.reduce_sum(rs_cur[:, :, g, i0:i0 + RT], eov,
                                         axis=mybir.AxisListType.X)
                    for r in range(RT):
                        nc.gpsimd.tensor_tensor(cs_cur[:, :, g, :],
                                                cs_cur[:, :, g, :],
                                                eov[:, :, r, :], OP.add)
                    if s == steps - 1:
                        for m2 in range(2):
                            nc.vector.tensor_scalar(eo[:, m2, :], eo[:, m2, :],
                                                    bvec("e_b3cum", m2), None,
                                                    OP.add)
                        nc.sync.dma_start(
                            edge_out.rearrange("(fh p) e -> p fh e", p=128)
                            [:, :, t * ET:(t + 1) * ET], eo[:])
                    else:
                        nc.sync.dma_start(edg_ap(nxt, t), eo[:])

                # sent/recv = delta of row/col sums (diagonal cancels exactly)
                nc.vector.tensor_tensor(sentT[:], rs_cur[:], rs_prev[:], OP.subtract)
                nc.vector.tensor_tensor(recvT[:], cs_cur[:], cs_prev[:], OP.subtract)
                nc.vector.reduce_sum(geT[:], sentT[:], axis=mybir.AxisListType.X)

                # node-MLP glob term [512, GPC]
                gwn = steptmp.tile([128, 4, GPC], F32, tag="gwn")
                for m in range(4):
                    pg_ = psum512(GPC)
                    _mm_chain(nc, pg_,
                              [ngwt[:, NGW_N1 + 3072 + kh * H + m * 128:
                                    NGW_N1 + 3072 + kh * H + m * 128 + 128]
                               for kh in range(2)],
                              [globT[:, kh, :] for kh in range(2)])
                    nc.scalar.activation(gwn[:, m, :], pg_, AF.Copy)

                # --- node MLP ---
                h1nd = misc.tile([128, 4, GPC * N], F32R, tag="h1n")
                rhs_n = [nodeT[:, 0, :], nodeT[:, 1, :],
                         sentT[:, 0].rearrange("p g n -> p (g n)"),
                         sentT[:, 1].rearrange("p g n -> p (g n)"),
                         recvT[:, 0].rearrange("p g n -> p (g n)"),
                         recvT[:, 1].rearrange("p g n -> p (g n)")]
                for m in range(4):
                    p1 = psum512(GPC * N)
                    _mm_chain(nc, p1,
                              [ngwt[:, NGW_N1 + kh * H + m * 128:
                                    NGW_N1 + kh * H + m * 128 + 128]
                               for kh in range(6)], rhs_n)
                    pv = p1.rearrange("p (g n) -> p g n", g=GPC)
                    nc.vector.tensor_tensor(
                        pv, pv,
                        gwn[:, m, :].unsqueeze(2).to_broadcast([128, GPC, N]),
                        OP.add)
                    nc.scalar.activation(h1nd[:, m, :], p1, AF.Relu,
                                         bias=bvec(f"s{s}_n_b1", m))
                h2nd = misc.tile([128, 4, GPC * N], F32R, tag="h2n")
                for m in range(4):
                    p2 = psum512(GPC * N)
                    _mm_chain(nc, p2,
                              [ngwt[:, NGW_N2 + kh * H + m * 128:
                                    NGW_N2 + kh * H + m * 128 + 128]
                               for kh in range(4)],
                              [h1nd[:, kh, :] for kh in range(4)])
                    nc.scalar.activation(h2nd[:, m, :], p2, AF.Relu,
                                         bias=bvec(f"s{s}_n_b2", m))
                n1t = misc.tile([128, 2, GPC * N], F32, tag="n1t")
                for m2 in range(2):
                    p3 = psum512(GPC * N)
                    _mm_chain(nc, p3,
                              [ngwt[:, NGW_N3 + kh * L + m2 * 128:
                                    NGW_N3 + kh * L + m2 * 128 + 128]
                               for kh in range(4)],
                              [h2nd[:, kh, :] for kh in range(4)])
                    nc.vector.tensor_scalar(n1t[:, m2, :], p3,
                                            bvec(f"s{s}_n_b3", m2), None, OP.add)
                nc.vector.reduce_sum(
                    gnT[:], n1t.rearrange("p fh (g n) -> p fh g n", g=GPC),
                    axis=mybir.AxisListType.X)

                # --- glob MLP ---
                rhs_g = [globT[:, 0, :], globT[:, 1, :], gnT[:, 0, :], gnT[:, 1, :],
                         geT[:, 0, :], geT[:, 1, :]]
                h1gd = misc.tile([128, 4, GPC], F32R, tag="h1g")
                for m in range(4):
                    p1 = psum512(GPC)
                    _mm_chain(nc, p1,
                              [ngwt[:, NGW_G1 + kh * H + m * 128:
                                    NGW_G1 + kh * H + m * 128 + 128]
                               for kh in range(6)], rhs_g)
                    nc.scalar.activation(h1gd[:, m, :], p1, AF.Relu,
                                         bias=bvec(f"s{s}_g_b1", m))
                h2gd = misc.tile([128, 4, GPC], F32R, tag="h2g")
                for m in range(4):
                    p2 = psum512(GPC)
                    _mm_chain(nc, p2,
                              [ngwt[:, NGW_G2 + kh * H + m * 128:
                                    NGW_G2 + kh * H + m * 128 + 128]
                               for kh in range(4)],
                              [h1gd[:, kh, :] for kh in range(4)])
                    nc.scalar.activation(h2gd[:, m, :], p2, AF.Relu,
                                         bias=bvec(f"s{s}_g_b2", m))
                g1t = misc.tile([128, 2, GPC], F32, tag="g1t")
                for m2 in range(2):
                    p3 = psum512(GPC)
                    _mm_chain(nc, p3,
                              [ngwt[:, NGW_G3 + kh * L + m2 * 128:
                                    NGW_G3 + kh * L + m2 * 128 + 128]
                               for kh in range(4)],
                              [h2gd[:, kh, :] for kh in range(4)])
                    nc.vector.tensor_scalar(g1t[:, m2, :], p3,
                                            bvec(f"s{s}_g_b3", m2), None, OP.add)

                # --- residuals ---
                nc.vector.tensor_tensor(nodeT[:], nodeT[:], n1t[:], OP.add)
                nc.vector.tensor_tensor(globT[:], globT[:], g1t[:], OP.add)

            # ---- outputs ----
            nc.sync.dma_start(
                node_out.rearrange("(fh p) n -> p fh n", p=128), nodeT[:])
            nc.sync.dma_start(
                glob_out.rearrange("(fh p) g -> p fh g", p=128), globT[:])

    nc.finalize()
    return nc


# ---------------------------------------------------------------------------
# host side
# ---------------------------------------------------------------------------


def _np(x):
    return np.asarray(x, dtype=np.float32)


def pack_weights(params, steps=STEPS):
    """Build the replicated weight arrays (everything except pos)."""
    pn = [( _np(l["w"]), _np(l["b"])) for l in params["posen"]]
    pe = [( _np(l["w"]), _np(l["b"])) for l in params["posee"]]
    pg = [( _np(l["w"]), _np(l["b"])) for l in params["poseg"]]
    gnn = [{k: [(_np(l["w"]), _np(l["b"])) for l in lp[k]]
            for k in ("edge", "node", "glob")} for lp in params["gnn"]]

    biasb = np.zeros((128, BIAS_COLS), np.float32)

    def put(name, v):
        c, w = BIAS_OFF[name]
        biasb[:, c:c + w] = v.reshape(w, 128).T

    put("pe_w1", pe[0][0][0])           # [512] first-layer weight row (in_dim=1)
    put("pe_b1", pe[0][1]); put("pe_b2", pe[1][1]); put("pe_b3", pe[2][1])
    put("pn_b1", pn[0][1]); put("pn_b2", pn[1][1]); put("pn_b3", pn[2][1])
    put("pg_b1", pg[0][1]); put("pg_b2", pg[1][1]); put("pg_b3", pg[2][1])

    # The edge-MLP output bias b3 is never added on-device (the e1 psum and
    # the stored edge state omit it).  All of its downstream effects are
    # folded into other biases here:
    #   - next step's edge-L1 bias gets W1e^T @ b3cum        (via eb1r rows)
    #   - node-L1 bias gets (Wn_s + Wn_r)^T @ (63*b3)        (sent/recv)
    #   - glob-L1 bias gets Wg_e^T @ (4032*b3)               (ge)
    #   - the final edge output adds b3cum explicitly         (e_b3cum)
    b3cum = np.zeros(L, np.float32)
    eb1r = np.zeros((steps, H), np.float32)
    for s in range(steps):
        e, n_, g_ = gnn[s]["edge"], gnn[s]["node"], gnn[s]["glob"]
        b3 = e[2][1]
        eb1r[s] = e[0][1] + b3cum @ e[0][0][:L]
        put(f"s{s}_e_b2", e[1][1])
        wn1, wg1 = n_[0][0], g_[0][0]
        put(f"s{s}_n_b1",
            n_[0][1] + (N - 1) * (b3 @ wn1[L:2 * L] + b3 @ wn1[2 * L:3 * L]))
        put(f"s{s}_n_b2", n_[1][1]); put(f"s{s}_n_b3", n_[2][1])
        put(f"s{s}_g_b1", g_[0][1] + N * (N - 1) * (b3 @ wg1[2 * L:]))
        put(f"s{s}_g_b2", g_[1][1]); put(f"s{s}_g_b3", g_[2][1])
        b3cum = b3cum + b3
    put("e_b3cum", b3cum)

    # L1 fold constants: col-selector rows + ones row (rows 65..72 unused)
    c65 = np.zeros((N + 9, ET), np.float32)
    for j in range(N):
        c65[j, :] = (np.arange(ET) % N == j)
    c65[N, :] = 1.0

    # posen masked weights
    w1n = pn[0][0]                       # [63, 512]
    wa = np.zeros((N, H), np.float32); wa[:N - 1] = w1n
    wb = np.zeros((N, H), np.float32); wb[1:] = w1n[:N - 1]
    maska = np.triu(np.ones((N, N), np.float32), 1)    # [j, i]: j < i
    maskb = np.tril(np.ones((N, N), np.float32), -1)   # [j, i]: j > i

    # poseg padded first layer [4096, 512]
    w1g = pg[0][0]
    wpad = np.zeros((NN, H), np.float32)
    idx_i = np.repeat(np.arange(N), N)
    idx_j = np.tile(np.arange(N), N)
    off = idx_i * (N - 1) + idx_j - (idx_j > idx_i)
    m = idx_i != idx_j
    wpad[np.arange(NN)[m]] = w1g[off[m]]

    prew = np.zeros((steps, 128, PREW_COLS), np.float32)
    ew = np.zeros((steps, 128, EW_COLS), np.float32)
    ngw = np.zeros((steps, 128, NGW_COLS), np.float32)
    for s in range(steps):
        e, n_, g_ = gnn[s]["edge"], gnn[s]["node"], gnn[s]["glob"]
        w1 = e[0][0]
        prew[s, :, PREW_R:PREW_R + 1024] = _chunked(w1[L:2 * L])
        prew[s, :, PREW_C:PREW_C + 1024] = _chunked(w1[2 * L:3 * L])
        prew[s, :, PREW_G:PREW_G + 1024] = _chunked(w1[3 * L:4 * L])
        ew[s, :, EW_1:EW_1 + 1024] = _chunked(w1[:L])
        ew[s, :, EW_2:EW_2 + 2048] = _chunked(e[1][0])
        ew[s, :, EW_3:EW_3 + 1024] = _chunked(e[2][0])
        wn1 = n_[0][0]
        ngw[s, :, NGW_N1:NGW_N1 + 3072] = _chunked(wn1[:3 * L])      # node|sent|recv
        ngw[s, :, NGW_N1 + 3072:NGW_N1 + 4096] = _chunked(wn1[3 * L:])  # glob part
        ngw[s, :, NGW_N2:NGW_N2 + 2048] = _chunked(n_[1][0])
        ngw[s, :, NGW_N3:NGW_N3 + 1024] = _chunked(n_[2][0])
        ngw[s, :, NGW_G1:NGW_G1 + 3072] = _chunked(g_[0][0])
        ngw[s, :, NGW_G2:NGW_G2 + 2048] = _chunked(g_[1][0])
        ngw[s, :, NGW_G3:NGW_G3 + 1024] = _chunked(g_[2][0])

    return {
        "maska": maska, "maskb": maskb, "biasb": biasb,
        "c65": c65, "eb1r": eb1r,
        "pna": wa, "pnb": wb,
        "pn2": _chunked(pn[1][0]), "pn3": _chunked(pn[2][0]),
        "pe2": _chunked(pe[1][0]), "pe3": _chunked(pe[2][0]),
        "pg1": _chunked(wpad), "pg2": _chunked(pg[1][0]),
        "pg3": _chunked(pg[2][0]),
        "prew": prew, "ew": ew, "ngw": ngw,
    }


_PROGRAM_CACHE = {}


TRACE = False          # set by test harness to capture a profile
LAST_RESULT = None


def kernel(conforms, params):
    global LAST_RESULT
    conforms = _np(conforms)
    wmap = pack_weights(params, STEPS)

    if STEPS not in _PROGRAM_CACHE:
        _PROGRAM_CACHE[STEPS] = build_program(STEPS)
    nc = _PROGRAM_CACHE[STEPS]

    in_maps = []
    for c in range(NCORES):
        posc = conforms[c * GPC:(c + 1) * GPC].reshape(GPC * N, 3).T
        in_maps.append({"pos": np.ascontiguousarray(posc), **wmap})

    res = run_bass_kernel_spmd(nc, in_maps, core_ids=list(range(NCORES)),
                               trace=TRACE)
    LAST_RESULT = res

    node = np.empty((B * N, L), np.float32)
    edge = np.empty((B * N * (N - 1), L), np.float32)
    glob = np.empty((B, L), np.float32)
    offd = ~np.eye(N, dtype=bool)
    EPG = N * (N - 1)
    for c in range(NCORES):
        r = res.results[c]
        node[c * GPC * N:(c + 1) * GPC * N] = r["node_out"].T
        glob[c * GPC:(c + 1) * GPC] = r["glob_out"].T
        ed = r["edge_out"].reshape(L, GPC, N, N).transpose(1, 2, 3, 0)
        for g in range(GPC):
            gi = c * GPC + g
            edge[gi * EPG:(gi + 1) * EPG] = ed[g][offd]
    return node, edge, glob
